# revision 28
# baseline (speedup 1.0000x reference)
"""Bahdanau-style attention kernel for Trainium2 (8 NeuronCores, batch-parallel).

Computes, for B=16, S=4096, H=512:
    hid  = hidden @ W_attn[:H] + b_attn                       (B, H)
    en   = tanh(hid[:,None,:] + enc @ W_attn[H:])             (B, S, H)
    lg   = en @ v                                             (B, S, 1)
    w    = softmax(lg, axis=1)
    ctx  = w^T @ enc                                          (B, 1, 2H)

Sharding: data-parallel over batch, 2 batches per core. Per core:
  - encT (e-major transpose of enc, prepared host-side during sharding)
    streams through the PE as the moving operand of the projection
    matmul. Default feed is fp16 (full PE rate, half HBM traffic);
    ATTN_FEED=f32r selects the 4-byte tf32-like mode instead.
  - tanh(+hid bias) fused on the scalar engine (per-partition bias).
  - logits computed with v replicated across 128 partitions as the
    stationary operand, so exp(logits) lands pre-broadcast for the
    context weighting; exp's accum_out yields the softmax normalizer.
  - context = sum_s w_s * enc[e, s] on the vector engine via
    scalar_tensor_tensor's fused accumulate (reduction along free dim).
  - softmax max-subtraction is replaced by a constant shift |v|_1 (a
    hard bound on |logit| since tanh in [-1,1]), exact after
    normalization.
No cross-core communication; output gathered on host.
"""

import os
import numpy as np
from contextlib import ExitStack

import concourse.bacc as bacc
import concourse.tile as tile
from concourse import mybir
from concourse.bass_utils import run_bass_kernel_spmd

F32 = mybir.dt.float32
F32R = mybir.dt.float32r
F16 = mybir.dt.float16

B, S, H = 16, 4096, 512
E = 2 * H                      # 1024 encoder feature dim
NCORES = 8
BPC = B // NCORES              # batches per core = 2
ET = E // 128                  # 8 e-tiles
HT = H // 128                  # 4 h-tiles
SBLK = 512                     # s-block width
NSB = S // SBLK                # 8 s-blocks per batch
KT = H // 128                  # 4 k-tiles for the hidden projection

FEED = os.environ.get("ATTN_FEED", "f16")    # "f16" | "f32r"
GROUP_SB = int(os.environ.get("ATTN_GROUP_SB", "2"))

TRACE = False          # set by test harness; harness-default off
LAST_RESULTS = None    # last BassKernelResults (for profiling in test.py)

_NC_CACHE = {}


def _feed_dt():
    return {"f32r": F32R, "f16": F16}[FEED]


def _feed_np(x):
    """Convert a float32 ndarray to the feed representation."""
    if FEED == "f32r":
        return np.ascontiguousarray(x, dtype=np.float32)
    return np.ascontiguousarray(x.astype(np.float16))


def _build():
    fdt = _feed_dt()
    nc = bacc.Bacc("TRN2", target_bir_lowering=False, debug=False)

    CW = (KT + 1) + KT * BPC            # bshift | hidT, packed (128, CW) f32
    encT = nc.dram_tensor("encT", [BPC, E, S], fdt, kind="ExternalInput").ap()
    We_d = nc.dram_tensor("We", [128, ET * H], fdt, kind="ExternalInput").ap()
    V_d = nc.dram_tensor("V128", [128, HT * 128], F16, kind="ExternalInput").ap()
    Wh_d = nc.dram_tensor("Wh16", [128, KT * H], F16, kind="ExternalInput").ap()
    cst_d = nc.dram_tensor("consts", [128, CW], F32, kind="ExternalInput").ap()
    ctx_d = nc.dram_tensor("ctx", [BPC, E], F32, kind="ExternalOutput").ap()

    with tile.TileContext(nc) as tc, ExitStack() as ctx:
        cpool = ctx.enter_context(tc.tile_pool(name="consts", bufs=1))
        epool = ctx.enter_context(tc.tile_pool(name="enc", bufs=3))
        tpool = ctx.enter_context(tc.tile_pool(name="tanh", bufs=2))
        wpool = ctx.enter_context(tc.tile_pool(name="wexp", bufs=2))
        jpool = ctx.enter_context(tc.tile_pool(name="junk", bufs=2))
        spool = ctx.enter_context(tc.tile_pool(name="stats", bufs=1))
        proj_bufs = 1 if GROUP_SB >= 4 else 2
        pp = ctx.enter_context(tc.tile_pool(name="pproj", bufs=proj_bufs, space="PSUM"))
        pl = ctx.enter_context(tc.tile_pool(name="plog", bufs=2, space="PSUM"))
        ph_pool = ctx.enter_context(tc.tile_pool(name="phid", bufs=1, space="PSUM"))

        # ---- PE warm-up: dummy matmuls while DMAs land (HAM -> K=8/8) ----
        wlhs = cpool.tile([128, 128], F16)
        wrhs = cpool.tile([128, 256], F16)
        nc.vector.memset(wlhs[:], 0.0)
        nc.vector.memset(wrhs[:], 0.0)
        wps = ph_pool.tile([128, 256], F32, name="warm", tag="ph")
        for _ in range(26):
            nc.tensor.matmul(wps[:], wlhs[:], wrhs[:], start=True, stop=True)

        # ---- constants: 3 packed DMAs (small gate-the-warmup one first) ----
        cst_sb = cpool.tile([128, CW], F32)
        nc.scalar.dma_start(cst_sb[:], cst_d)
        We_sb = cpool.tile([128, ET * H], fdt)          # per e-tile: (128, 512)
        nc.scalar.dma_start(We_sb[:, 0:2 * H], We_d[:, 0:2 * H])
        Wh_sb = cpool.tile([128, KT * H], F16)
        nc.scalar.dma_start(Wh_sb[:], Wh_d)
        nc.scalar.dma_start(We_sb[:, 2 * H:], We_d[:, 2 * H:])
        V_sb = cpool.tile([128, HT * 128], F16)
        nc.scalar.dma_start(V_sb[:], V_d)
        bsh_sb = cst_sb[:, 0:KT + 1]
        hidT16 = cpool.tile([128, KT * BPC], F16)
        nc.vector.tensor_copy(hidT16[:], cst_sb[:, KT + 1:KT + 1 + KT * BPC])

        # ---- hidden projection: hid_sb[:, h*BPC + b] = (hidden @ Wh + b)[b, h-tile]
        hid_sb = spool.tile([128, HT * BPC], F32)
        for h in range(HT):
            ph = ph_pool.tile([128, BPC], F32, name="ph")
            for k in range(KT):
                nc.tensor.matmul(
                    ph[:],
                    Wh_sb[:, k * H + h * 128: k * H + (h + 1) * 128],
                    hidT16[:, k * BPC:(k + 1) * BPC],
                    start=(k == 0), stop=(k == KT - 1),
                )
            nc.vector.tensor_scalar_add(
                hid_sb[:, h * BPC:(h + 1) * BPC], ph[:], bsh_sb[:, h:h + 1])

        # ---- stats accumulators ----
        zslots = spool.tile([128, BPC * NSB], F32)
        cslots = spool.tile([128, BPC * ET * NSB], F32)
        nc.vector.memset(cslots[:], 0.0)

        ctx_red = spool.tile([128, BPC * ET], F32)
        zred = spool.tile([128, BPC], F32)
        zrec = spool.tile([128, BPC], F32)
        ctx_fin = spool.tile([128, BPC * ET], F32)

        groups = []
        pos = 0
        while pos < NSB - 2:
            groups.append(list(range(pos, pos + GROUP_SB)))
            pos += GROUP_SB
        while pos < NSB:
            groups.append([pos])
            pos += 1
        ngrp = len(groups)
        GW = GROUP_SB * SBLK
        for b in range(BPC):
            for g, sbs in enumerate(groups):
                gsb = len(sbs)
                gw = gsb * SBLK
                g0 = sbs[0]
                encg = epool.tile([128, ET * GW], fdt, name="encg",
                                  tag="encg")
                # group 0 of batch 0 lands in finer slices so the first
                # matmuls can start as early as possible
                chunks = [2, 2, 4] if (b == 0 and g == 0) else [4, 4]
                et0 = 0
                for nt in chunks:
                    nc.sync.dma_start(
                        encg[:, et0 * gw:(et0 + nt) * gw].rearrange(
                            "p (t s) -> p t s", t=nt),
                        encT[b].rearrange("(t p) s -> p t s", p=128)[
                            :, et0:et0 + nt,
                            g0 * SBLK:g0 * SBLK + gw],
                    )
                    et0 += nt

                # big projection + tanh, h-tile at a time
                tanh_t = {}
                for h in range(HT):
                    proj = {}
                    for i in range(gsb):
                        proj[i] = pp.tile([128, SBLK], F32, name=f"proj_{i}")
                    for e in range(ET):
                        lhs = We_sb[:, e * H + h * 128: e * H + (h + 1) * 128]
                        for i in range(gsb):
                            nc.tensor.matmul(
                                proj[i][:], lhs,
                                encg[:, e * gw + i * SBLK:
                                     e * gw + (i + 1) * SBLK],
                                start=(e == 0), stop=(e == ET - 1),
                            )
                    for i in range(gsb):
                        tt = tpool.tile([128, SBLK], F16, name=f"tanh_{h}_{i}")
                        nc.scalar.activation(
                            tt[:], proj[i][:], mybir.ActivationFunctionType.Tanh,
                            bias=hid_sb[:, h * BPC + b: h * BPC + b + 1],
                        )
                        tanh_t[(h, i)] = tt

                # logits (broadcast across partitions) + exp + Z accum
                wg = wpool.tile([128, GW], F32, name="wg")[:, 0:gw]
                lg = {}
                for i in range(gsb):
                    lg[i] = pl.tile([128, SBLK], F32, name=f"logits_{i}",
                                    bufs=1)
                for h in range(HT):
                    for i in range(gsb):
                        nc.tensor.matmul(
                            lg[i][:], V_sb[:, h * 128:(h + 1) * 128],
                            tanh_t[(h, i)][:],
                            start=(h == 0), stop=(h == HT - 1),
                        )
                for i, sb in enumerate(sbs):
                    nc.scalar.activation(
                        wg[:, i * SBLK:(i + 1) * SBLK], lg[i][:],
                        mybir.ActivationFunctionType.Exp,
                        bias=bsh_sb[:, KT:KT + 1],
                        accum_out=zslots[:, b * NSB + sb: b * NSB + sb + 1],
                    )

                # context accumulation on DVE: one fused op per e-tile.
                # Final group runs at per-s-block granularity so the tail
                # chain behind the last matmuls is as short as possible.
                splits = [(0, gw, sbs[0])] if gsb > 1 else \
                    [(i * SBLK, SBLK, sbs[i]) for i in range(gsb)]
                for off, width, sbcol in splits:
                    for e in range(ET):
                        jt = jpool.tile([128, GW], F16, name="junk")
                        src = encg[:, e * gw + off:e * gw + off + width]
                        if FEED == "f32r":
                            src = src.bitcast(F32)
                        col = (b * ET + e) * NSB + sbcol
                        nc.vector.scalar_tensor_tensor(
                            jt[:, 0:width], src, 1.0, wg[:, off:off + width],
                            mybir.AluOpType.mult, mybir.AluOpType.mult,
                            accum_out=cslots[:, col:col + 1],
                        )

            # finalize this batch: ctx = (sum_sb ctx_partial) / Z
            nc.vector.tensor_reduce(
                ctx_red[:, b * ET:(b + 1) * ET],
                cslots[:, b * ET * NSB:(b + 1) * ET * NSB].rearrange(
                    "p (q s) -> p q s", s=NSB),
                axis=mybir.AxisListType.X, op=mybir.AluOpType.add)
            nc.vector.tensor_reduce(
                zred[:, b:b + 1],
                zslots[:, b * NSB:(b + 1) * NSB].rearrange(
                    "p (q s) -> p q s", s=NSB),
                axis=mybir.AxisListType.X, op=mybir.AluOpType.add)
            nc.vector.reciprocal(zrec[:, b:b + 1], zred[:, b:b + 1])
            nc.vector.tensor_scalar_mul(
                ctx_fin[:, b * ET:(b + 1) * ET],
                ctx_red[:, b * ET:(b + 1) * ET], zrec[:, b:b + 1])
            nc.sync.dma_start(
                ctx_d[b].rearrange("(e p) -> p e", p=128),
                ctx_fin[:, b * ET:(b + 1) * ET])

    nc.compile()
    return nc


def kernel(hidden, encoder_outputs, W_attn, b_attn, v):
    global LAST_RESULTS
    hidden = np.asarray(hidden, dtype=np.float32)
    encoder_outputs = np.asarray(encoder_outputs, dtype=np.float32)
    W_attn = np.asarray(W_attn, dtype=np.float32)
    b_attn = np.asarray(b_attn, dtype=np.float32)
    v = np.asarray(v, dtype=np.float32)

    key = (FEED, GROUP_SB)
    if key not in _NC_CACHE:
        _NC_CACHE[key] = _build()
    nc = _NC_CACHE[key]

    # SBUF-layout packed constants (partition dim = 128 rows)
    We_f = _feed_np(
        W_attn[H:].reshape(ET, 128, H).transpose(1, 0, 2).reshape(128, ET * H))
    V128 = np.ascontiguousarray(np.broadcast_to(
        v.reshape(HT, 128, 1).transpose(1, 0, 2), (128, HT, 128)
    ).reshape(128, HT * 128).astype(np.float16))
    Wh16 = np.ascontiguousarray(W_attn[:H].reshape(KT, 128, H).transpose(
        1, 0, 2).reshape(128, KT * H).astype(np.float16))
    shift = float(np.abs(v).sum())
    bsh = np.zeros((128, KT + 1), dtype=np.float32)
    bsh[:, :KT] = b_attn.reshape(KT, 128).T
    bsh[:, KT] = -shift

    in_maps = []
    for c in range(NCORES):
        sl = slice(c * BPC, (c + 1) * BPC)
        encT = _feed_np(encoder_outputs[sl].transpose(0, 2, 1))
        hidT_pack = np.ascontiguousarray(
            hidden[sl].T.reshape(KT, 128, BPC).transpose(1, 0, 2)
        ).reshape(128, KT * BPC)
        consts = np.ascontiguousarray(
            np.concatenate([bsh, hidT_pack], axis=1, dtype=np.float32))
        in_maps.append({
            "encT": encT, "We": We_f, "V128": V128, "consts": consts,
            "Wh16": Wh16,
        })

    res = run_bass_kernel_spmd(
        nc, in_maps, core_ids=list(range(NCORES)), trace=TRACE)
    LAST_RESULTS = res

    out = np.empty((B, 1, E), dtype=np.float32)
    for c in range(NCORES):
        out[c * BPC:(c + 1) * BPC, 0, :] = res.results[c]["ctx"]
    return out
